# revision 36
# baseline (speedup 1.0000x reference)
"""MLA attention Trainium2 kernel: nn_MultiHeadLatentAttention_31722628448847.

Full computation (B=1, T=2048, C=2048, H=16, G=4, Dl=32):
  q  = x @ Wq.T   -> [T, H, G, Dl]
  lk = x @ Wlk.T  -> [T, H, Dl]
  lv = x @ Wlv.T  -> [T, H, Dl]
  scores[h,g,t,s] = (q[t,h,g,:] . lk[s,h,:]) / sqrt(128)
  probs = softmax_s(scores)
  attn[t, h,g,:] = sum_s probs * lv[s,h,:]
  out = attn @ Wo.T

Sharding: 2 heads per core (8 cores); each core computes a full-width
partial of the output projection; partials are summed on the host.

Device layout is fully transposed ("feature on partitions"): the host
passes xT [C, T] and pre-transposed weight shards, all cast to bf16.

Structure (v2):
  - single merged projection weight W_all = [Wq; Wlk; Wlv] (384 rows)
    iterated kb-outer so compute overlaps the xT DMA stream
  - scoresT [s, t] via 4-way row-tiled K=32 matmuls issued as one
    4-burst, then two [128,1024] Exp activations (fused 1/sqrt(128))
  - AV matmuls consume expT with K=s=128; an extra all-ones lhsT column
    produces softmax denominators; (g,h) issue order alternates PSUM
    bank and PE column-group every matmul
  - normalization: DVE copy + reciprocal_approx_fast + gpsimd
    partition_broadcast + DVE multiply (no PE, no ScalarE)
  - output projection accumulates into the (freed) AV PSUM banks and
    DMAs PSUM->DRAM directly, so the score banks stay free and Exp
    runs continuously across t-chunks
"""

import numpy as np

T = 2048
C = 2048
HEADS_PER_CORE = 2
DH = 128  # head dim (q)
DL = 32  # latent dim
G = 4  # latent sub-heads per head
N_CORES = 8
TC = 512  # t-chunk (matmul free dim)
SCALE = 1.0 / np.sqrt(np.float32(DH))
DVE_EXP_SHARE = False  # offload 25% of exp to DVE (Schraudolph)
WROWS = HEADS_PER_CORE * DH + 2 * HEADS_PER_CORE * DL  # 384


def build_program(t=T, c=C):
    import concourse.mybir as mybir
    import concourse.tile as tile
    from concourse import bacc

    bf16 = mybir.dt.bfloat16
    f32 = mybir.dt.float32

    nc = bacc.Bacc("TRN2", target_bir_lowering=False, debug=False, num_devices=1)

    xT_d = nc.dram_tensor("xT", [c, t], bf16, kind="ExternalInput").ap()
    wT_d = nc.dram_tensor("wT", [c, WROWS], bf16, kind="ExternalInput").ap()
    woT_d = nc.dram_tensor("woT", [HEADS_PER_CORE * DH, c], bf16, kind="ExternalInput").ap()
    ident_d = nc.dram_tensor("ident", [128, 128], bf16, kind="ExternalInput").ap()
    out_d = nc.dram_tensor("out", [t, c], bf16, kind="ExternalOutput").ap()

    with tile.TileContext(nc) as tc_:
        _emit(nc, tc_, tile, mybir, bf16, f32, xT_d, wT_d, woT_d, ident_d, out_d, t, c)
    nc.compile()
    return nc


def _emit(nc, tc_, tile, mybir, bf16, f32, xT_d, wT_d, woT_d, ident_d, out_d, t, c):
    from contextlib import ExitStack

    from concourse import library_config

    EXP = mybir.ActivationFunctionType.Exp
    H = HEADS_PER_CORE
    n_cb = c // 128  # contraction blocks for projections
    n_sb = t // 128  # s-blocks
    n_tc = t // TC  # t-chunks
    n_tb = TC // 128  # t-blocks per chunk
    QCOLS = H * DH  # 256
    KCOLS = H * DL  # 64
    n_mb = WROWS // 128  # 3 projection M-blocks

    # gpsimd library with partition_broadcast
    nc.gpsimd.load_library(library_config.attn)

    ctx = ExitStack()
    with ctx:
        # ---------------- persistent SBUF inputs ----------------
        wpool = ctx.enter_context(tc_.tile_pool(name="wpool", bufs=1))
        # xT in two half-t waves so wave-0 compute overlaps wave-1 DMA
        xTh = [[None] * n_cb for _ in range(2)]
        wT_sb = []
        for kb in range(n_cb):
            w = wpool.tile([128, WROWS], bf16, name=f"wT{kb}")
            nc.sync.dma_start(w[:], wT_d[kb * 128 : (kb + 1) * 128, :])
            wT_sb.append(w)
            xa = wpool.tile([128, t // 2], bf16, name=f"xTa{kb}")
            nc.sync.dma_start(xa[:], xT_d[kb * 128 : (kb + 1) * 128, 0 : t // 2])
            xTh[0][kb] = xa
        for kb in range(n_cb):
            xb = wpool.tile([128, t // 2], bf16, name=f"xTb{kb}")
            nc.sync.dma_start(xb[:], xT_d[kb * 128 : (kb + 1) * 128, t // 2 : t])
            xTh[1][kb] = xb
        woT_sb = []
        for h in range(H):
            wo = wpool.tile([128, c], bf16, name=f"woT{h}")
            nc.sync.dma_start(wo[:], woT_d[h * 128 : (h + 1) * 128, :])
            woT_sb.append(wo)
        ident = wpool.tile([128, 128], bf16, name="ident")
        nc.sync.dma_start(ident[:], ident_d[:, :])

        # ---------------- projection outputs (SBUF) ----------------
        apool = ctx.enter_context(tc_.tile_pool(name="apool", bufs=1))
        qT = [apool.tile([128, t], bf16, name=f"qT{h}") for h in range(H)]
        lkT = [apool.tile([128, t], bf16, name=f"lkT{h}") for h in range(H)]
        # lv natural layout per s-block: [128 s, 66]: cols 0-31 lv_h0,
        # col 32 ones, cols 33-64 lv_h1, col 65 ones
        lv_all = apool.tile([128, 66 * n_sb], bf16, name="lv_all")
        lv_sb = [lv_all[:, 66 * sb : 66 * (sb + 1)] for sb in range(n_sb)]
        lvT_tmp = apool.tile([KCOLS, t], bf16, name="lvT_tmp")

        # ---------------- projections (kb-outer, 2 waves) ----------------
        for sb in range(n_sb):
            nc.vector.memset(lv_sb[sb][:, DL : DL + 1], 1.0)
            nc.vector.memset(lv_sb[sb][:, 2 * DL + 1 : 2 * DL + 2], 1.0)
        pctx = ExitStack()
        ppool = pctx.enter_context(tc_.tile_pool(name="ppool", bufs=1, space="PSUM"))
        tpool = pctx.enter_context(tc_.tile_pool(name="tpool", bufs=2, space="PSUM"))
        for w in range(2):
            pss = [
                [
                    ppool.tile([128, TC], f32, name=f"ps{mb}_{i}", tag=f"pp{mb}{i}")
                    for i in range(2)
                ]
                for mb in range(n_mb)
            ]
            for kb in range(n_cb):
                for mb in range(n_mb):
                    for i in range(2):
                        nc.tensor.matmul(
                            pss[mb][i][:],
                            wT_sb[kb][:, mb * 128 : (mb + 1) * 128],
                            xTh[w][kb][:, i * TC : (i + 1) * TC],
                            start=(kb == 0),
                            stop=(kb == n_cb - 1),
                        )
            for i in range(2):
                tsl = slice((2 * w + i) * TC, (2 * w + i + 1) * TC)
                for h in range(H):
                    # q rows: mb = h
                    nc.vector.tensor_copy(qT[h][:, tsl], pss[h][i][:])
                    # lk rows: mb2 rows h*32..h*32+32 -> replicate to 4 g-strips
                    for g in range(G):
                        nc.vector.tensor_copy(
                            lkT[h][g * DL : (g + 1) * DL, tsl],
                            pss[2][i][h * DL : (h + 1) * DL, :],
                        )
                # lv rows: mb2 rows 64..127
                nc.vector.tensor_copy(
                    lvT_tmp[:, tsl], pss[2][i][KCOLS : 2 * KCOLS, :]
                )
                # lv PE-transpose for this chunk's s-blocks
                for sb in range((2 * w + i) * 4, (2 * w + i + 1) * 4):
                    pt = tpool.tile([128, KCOLS], bf16, name="ps_t", tag="ps")
                    nc.tensor.transpose(
                        pt[:],
                        lvT_tmp[:, sb * 128 : (sb + 1) * 128],
                        ident[0:KCOLS, 0:KCOLS],
                    )
                    nc.vector.tensor_copy(lv_sb[sb][:, 0:DL], pt[:, 0:DL])
                    nc.vector.tensor_copy(
                        lv_sb[sb][:, DL + 1 : 2 * DL + 1], pt[:, DL : 2 * DL]
                    )
        pctx.close()

        # ---------------- attention + output projection ----------------
        # PSUM: sc 2 x [128,1024] (4 banks) + av/out 4 x [128,512] (4 banks)
        scpool = ctx.enter_context(tc_.tile_pool(name="scpool", bufs=2, space="PSUM"))
        avpool = ctx.enter_context(tc_.tile_pool(name="avpool", bufs=G, space="PSUM"))
        expool = ctx.enter_context(tc_.tile_pool(name="expool", bufs=14))
        e32pool = ctx.enter_context(tc_.tile_pool(name="e32pool", bufs=1))
        atpool = ctx.enter_context(tc_.tile_pool(name="atpool", bufs=2 * H))
        npool = ctx.enter_context(tc_.tile_pool(name="npool", bufs=4))
        bcpool = ctx.enter_context(tc_.tile_pool(name="bcpool", bufs=G * H))
        opool = ctx.enter_context(tc_.tile_pool(name="opool", bufs=4))

        # Schraudolph fast-exp in bf16-space for the DVE share of the
        # softmax: bits16 = int16(A*score + B); bitcast to bf16
        # approximates exp(score*SCALE) with max rel err ~3.3%.
        SCHRA_A = float(2.0**7 / np.log(2.0) * SCALE)
        SCHRA_B = float(127 * 2**7 - 5.25)
        i16 = mybir.dt.int16

        exp_cache = {}

        def emit_scores_exp(tci, sb):
            tsl = slice(tci * TC, (tci + 1) * TC)
            exp_ts = []
            exp32s = []
            for h in range(H):
                use_dve = DVE_EXP_SHARE and sb % 2 == 0
                exp_t = expool.tile([128, G * TC], bf16, name="exp_t", tag="ex")
                exp_ts.append(exp_t)
                scs = [
                    scpool.tile([128, 2 * TC], f32, name=f"sc{gp}", tag="sc")
                    for gp in range(2)
                ]
                for g in range(G):
                    nc.tensor.matmul(
                        scs[g // 2][:, (g % 2) * TC : (g % 2 + 1) * TC],
                        lkT[h][g * DL : (g + 1) * DL, sb * 128 : (sb + 1) * 128],
                        qT[h][g * DL : (g + 1) * DL, tsl],
                        start=True,
                        stop=True,
                        tile_position=(g * DL, 0),
                    )
                nc.scalar.activation(
                    exp_t[:, 0 : 2 * TC], scs[0][:], EXP, scale=float(SCALE)
                )
                if use_dve:
                    e16 = e32pool.tile([128, 2 * TC], i16, name="e16", tag="e16")
                    nc.vector.tensor_scalar(
                        e16[:],
                        scs[1][:],
                        SCHRA_A,
                        SCHRA_B,
                        mybir.AluOpType.mult,
                        mybir.AluOpType.add,
                    )
                    exp32s.append(e16)
                else:
                    nc.scalar.activation(
                        exp_t[:, 2 * TC : 4 * TC], scs[1][:], EXP, scale=float(SCALE)
                    )
                    exp32s.append(None)
            return exp_ts, exp32s

        PREFETCH_SB = 5

        for tci in range(n_tc):
            # AV accumulators: one [128, TC] bank per group;
            # rows 0-31 attnU_h0, 32 denom_h0, 64-95 attnU_h1, 96 denom_h1
            av = [avpool.tile([128, TC], f32, name=f"av{g}", tag="av") for g in range(G)]
            for g in range(G):
                nc.vector.memset(av[g][:], 0.0)
            for sb in range(n_sb):
                if (tci, sb) in exp_cache:
                    exp_ts, exp32s = exp_cache.pop((tci, sb))
                else:
                    exp_ts, exp32s = emit_scores_exp(tci, sb)
                # AV: alternate PSUM bank (g) and PE column group (h)
                for i, (g, h) in enumerate(
                    [(0, 0), (1, 1), (2, 0), (3, 1), (0, 1), (1, 0), (2, 1), (3, 0)]
                ):
                    last = sb == n_sb - 1 and i >= 4
                    e16 = exp32s[h]
                    if g >= 2 and e16 is not None:
                        rhs = e16[:, (g % 2) * TC : (g % 2 + 1) * TC].bitcast(bf16)
                    else:
                        rhs = exp_ts[h][:, g * TC : (g + 1) * TC]
                    nc.tensor.matmul(
                        av[g][h * 64 : h * 64 + DL + 1, :],
                        lv_sb[sb][:, h * (DL + 1) : (h + 1) * (DL + 1)],
                        rhs,
                        start=False,
                        stop=last,
                        skip_group_check=True,
                        tile_position=(0, h * 64),
                    )
            # normalize -> attnT (bf16) per head for this t-chunk
            at = [atpool.tile([128, TC], bf16, name=f"at{h}", tag="at") for h in range(H)]
            # h-major so at[h0] completes first and the out-projection's
            # h0 matmuls can start while h1 still normalizes
            for h in range(H):
                bcs = {}
                for g in range(G):
                    # reciprocal_approx_fast requires a partition-base-0
                    # input; stage the denominator row through SBUF first
                    # den row is ready as soon as AV stops (early-ready),
                    # so ScalarE can stage it without FIFO head-blocking
                    den = npool.tile([1, TC], f32, name="den", tag="den")
                    nc.scalar.copy(
                        den[:], av[g][h * 64 + DL : h * 64 + DL + 1, :]
                    )
                    rec = npool.tile([1, TC], f32, name="rec", tag="rec")
                    nc.vector.reciprocal_approx_fast(rec[:], den[:])
                    bc = bcpool.tile([DL, TC], f32, name="bc", tag="bc")
                    nc.gpsimd.partition_broadcast(bc[:], rec[:], channels=DL)
                    bcs[g] = bc
                for g in range(G):
                    nc.vector.tensor_tensor(
                        at[h][g * DL : (g + 1) * DL, :],
                        av[g][h * 64 : h * 64 + DL, :],
                        bcs[g][:],
                        mybir.AluOpType.mult,
                    )
            # prefetch next chunk's first score/exp units so ScalarE
            # stays busy while the out-projection waits on the norm chain
            if tci + 1 < n_tc:
                for psb in range(PREFETCH_SB):
                    exp_cache[(tci + 1, psb)] = emit_scores_exp(tci + 1, psb)
            # output projection for this t-chunk: accumulate into the
            # (now free) av banks, stage bf16 in SBUF, DMA out
            for tb in range(n_tb):
                t0 = tci * TC + tb * 128
                wos = [
                    avpool.tile([128, TC], f32, name=f"wo_ps{oc}", tag="av")
                    for oc in range(c // TC)
                ]
                for h in range(H):
                    for oc in range(c // TC):
                        nc.tensor.matmul(
                            wos[oc][:],
                            at[h][:, tb * 128 : (tb + 1) * 128],
                            woT_sb[h][:, oc * TC : (oc + 1) * TC],
                            start=(h == 0),
                            stop=(h == H - 1),
                        )
                for oc in range(c // TC):
                    ot = opool.tile([128, TC], bf16, name="ot", tag="ot")
                    nc.vector.tensor_copy(ot[:], wos[oc][:])
                    nc.sync.dma_start(
                        out_d[t0 : t0 + 128, oc * TC : (oc + 1) * TC], ot[:]
                    )


# ---------------- host side ----------------


def shard_inputs(x, Wq, Wlk, Wlv, Wo):
    """Returns per-core input dicts (bf16, pre-transposed)."""
    import ml_dtypes

    bf = ml_dtypes.bfloat16
    X = np.ascontiguousarray(x.reshape(-1, x.shape[-1]))  # [T, C]
    xT = np.ascontiguousarray(X.T).astype(bf)
    ident = np.eye(128, dtype=np.float32).astype(bf)
    maps = []
    for core in range(N_CORES):
        h0 = core * HEADS_PER_CORE
        qr = slice(h0 * DH, (h0 + HEADS_PER_CORE) * DH)
        kr = slice(h0 * DL, (h0 + HEADS_PER_CORE) * DL)
        W_all = np.concatenate([Wq[qr, :], Wlk[kr, :], Wlv[kr, :]], axis=0)
        maps.append(
            {
                "xT": xT,
                "wT": np.ascontiguousarray(W_all.T).astype(bf),
                "woT": np.ascontiguousarray(Wo[:, qr].T).astype(bf),
                "ident": ident,
            }
        )
    return maps


_CACHE = {}


def kernel(x, Wq, Wk, Wv, Wlk, Wlv, Wo):
    """Full-input entry point. Wk/Wv are unused by the reference forward."""
    if "nc" not in _CACHE:
        _CACHE["nc"] = build_program()
    nc = _CACHE["nc"]
    from concourse.bass_utils import run_bass_kernel_spmd

    in_maps = shard_inputs(
        np.asarray(x, dtype=np.float32),
        np.asarray(Wq, dtype=np.float32),
        np.asarray(Wlk, dtype=np.float32),
        np.asarray(Wlv, dtype=np.float32),
        np.asarray(Wo, dtype=np.float32),
    )
    res = run_bass_kernel_spmd(nc, in_maps, list(range(N_CORES)))
    out = np.zeros((T, C), dtype=np.float32)
    for r in res.results:
        out += np.asarray(r["out"], dtype=np.float32)
    return out.reshape(1, T, C)


def _cache_get():
    return _CACHE["nc"]
